# revision 1
# baseline (speedup 1.0000x reference)
"""Trainium2 Bass kernel for nn_BOJANET_23545010717406.

Pipeline (per core, batch=64 of 512, data-parallel over 8 cores):
  x -> [im2col DMA] -> FIR matmul (block-diag, 4 position-blocks) -> demod
  (s = I^2+Q^2, inv = exp(-0.5 ln s), mag = s*inv, cos = I*inv, sin = Q*inv)
  -> gated RNN over S=8192 steps (one 25x24 matmul + sigmoid + 4 DVE ops per
  step; tanh folded into sigmoid via tanh(x) = 2*sigmoid(2x)-1)
  -> phase rotation products -> output matmul -> y.

Self-contained: only imports from /opt/trn_rl_repo (system toolchain).
"""

import sys
import numpy as np

sys.path.insert(0, "/opt/trn_rl_repo")

import concourse.bass as bass  # noqa: E402
import concourse.mybir as mybir  # noqa: E402
import concourse.tile as tile  # noqa: E402

F32 = mybir.dt.float32
F32R = mybir.dt.float32r
AF = mybir.ActivationFunctionType
OP = mybir.AluOpType

# problem constants
B_TOT, S, VD, H, WIN = 512, 8192, 6, 12, 16
NCORES = 8
B = B_TOT // NCORES          # 64 batch rows per core
T = 64                       # timesteps per chunk
NCH = S // T                 # chunks
NB = 4                       # position blocks (time-split of a chunk)
TB = T // NB                 # timesteps per block (16)
SUBT = 8                     # timesteps per block covered by one matmul
NSUB = TB // SUBT            # matmul sub-chunks per chunk (2)
NSUBF = SUBT * B             # moving columns per FIR matmul (512)
CW = TB * B                  # per-chunk per-block column width (1024)

_counter = [0]


def _ap(base, offset, dims):
    """Hand-crafted access pattern: dims = [(step, count), ...] (elements)."""
    a = base.copy()
    v = a.ap
    v.clear()
    for st, cnt in dims:
        v.append([int(st), int(cnt)])
    a.offset = int(offset)
    return a


def _pitch(t):
    return t[:].ap[0][0]


def split_waits(nc, max_inline=1):
    """Hoist excess sem waits into standalone event-sem instructions.

    The neuronxcc walrus here accepts at most one sync-wait per
    instruction; Tile can emit several. Moving waits to preceding
    instructions on the same engine is semantically identical (engine
    queues are in-order).
    """
    n = 0
    for fn in nc.m.functions:
        for blk in fn.blocks:
            out = []
            changed = False
            for ins in blk.instructions:
                si = ins.sync_info
                waits = list(si.on_wait) if si is not None else []
                cap = 2 if type(ins).__name__ == "InstEventSemaphore" else max_inline
                if len(waits) > cap:
                    changed = True
                    extra, keep = waits[:-cap], waits[-cap:]
                    for i in range(0, len(extra), 2):
                        _counter[0] += 1
                        ev = mybir.InstEventSemaphore(
                            name=f"WSPL-{_counter[0]}", ins=[], outs=[])
                        ev.engine = ins.engine
                        ev.sync_info = mybir.SyncInfo(
                            on_wait=extra[i:i + 2], on_update=[])
                        out.append(ev)
                        n += 1
                    ins.sync_info = mybir.SyncInfo(
                        on_wait=keep, on_update=list(si.on_update))
                out.append(ins)
            if changed:
                blk.instructions = out
    return n


def derive_weights(inp):
    """Host-side packing of all stationary matmul operands (numpy, fp32)."""
    WI = np.asarray(inp["fir_I_w"], np.float32)     # (VD, WIN)
    WQ = np.asarray(inp["fir_Q_w"], np.float32)
    W_fi = np.asarray(inp["W_fi_w"], np.float32)    # (H, 2VD)
    b_fi = np.asarray(inp["W_fi_b"], np.float32)    # (H,)
    W_fh = np.asarray(inp["W_fh_w"], np.float32)    # (H, H)
    W_gi = np.asarray(inp["W_gi_w"], np.float32)
    b_gi = np.asarray(inp["W_gi_b"], np.float32)
    W_gh = np.asarray(inp["W_gh_w"], np.float32)
    WoI = np.asarray(inp["W_out_I_w"], np.float32)  # (1, H)
    bI = float(np.asarray(inp["W_out_I_b"], np.float32)[0])
    WoQ = np.asarray(inp["W_out_Q_w"], np.float32)
    bQ = float(np.asarray(inp["W_out_Q_b"], np.float32)[0])

    # FIR: rows p = pair*64 + blk*16 + k  (consecutive windows: TB == WIN)
    # cols: blk*6+u -> I_fir[u], 32+blk*6+u -> Q_fir[u] (32-aligned base)
    lhsT_fir = np.zeros((128, 56), np.float32)
    for blk in range(NB):
        for u in range(VD):
            for k in range(WIN):
                rI = blk * 16 + k          # window of I signal
                rQ = 64 + blk * 16 + k     # window of Q signal
                lhsT_fir[rI, blk * 6 + u] = WI[u, k]
                lhsT_fir[rQ, blk * 6 + u] = -WQ[u, k]
                lhsT_fir[rI, 32 + blk * 6 + u] = WQ[u, k]
                lhsT_fir[rQ, 32 + blk * 6 + u] = WI[u, k]

    # Recurrence: psum rows j<12 = f_pre[j]; rows 32+j = 2*g_pre[j]
    # rhs rows: 0-11 h, 12-23 L = [mag(6); mag2(6)], 24 ones
    lhsT_rec = np.zeros((25, 44), np.float32)
    lhsT_rec[0:12, 0:12] = W_fh.T
    lhsT_rec[0:12, 32:44] = 2.0 * W_gh.T
    lhsT_rec[12:24, 0:12] = W_fi.T
    lhsT_rec[12:24, 32:44] = 2.0 * W_gi.T
    lhsT_rec[24, 0:12] = b_fi
    lhsT_rec[24, 32:44] = 2.0 * b_gi

    # Output: rhs rows 0-47 X_I (blk*12+ch), 64-111 X_Q, 112 ones
    # out rows blk*2 + oc;  out0 = a-b, out1 = b+a
    lhsT_out = np.zeros((113, 8), np.float32)
    for blk in range(NB):
        for ch in range(H):
            lhsT_out[blk * 12 + ch, blk * 2 + 0] = WoI[0, ch]
            lhsT_out[64 + blk * 12 + ch, blk * 2 + 0] = -WoQ[0, ch]
            lhsT_out[blk * 12 + ch, blk * 2 + 1] = WoI[0, ch]
            lhsT_out[64 + blk * 12 + ch, blk * 2 + 1] = WoQ[0, ch]
        lhsT_out[112, blk * 2 + 0] = bI - bQ
        lhsT_out[112, blk * 2 + 1] = bI + bQ

    return {"lhsT_fir_I": np.ascontiguousarray(lhsT_fir[:, 0:24]),
            "lhsT_fir_Q": np.ascontiguousarray(lhsT_fir[:, 32:56]),
            "lhsT_rec": lhsT_rec, "lhsT_out": lhsT_out,
            "ones_row": np.ones((1, (T + 1) * B), np.float32),
            "zeros12": np.zeros((12, B), np.float32)}


def build_nc(s_len=S):
    """Emit the full Bass program for one core (batch B, seq s_len)."""
    nch = s_len // T
    nc = bass.Bass(num_swdge_queues=4)
    # xt: host-transposed input, (pair, t, b) with batch contiguous
    x_d = nc.declare_dram_parameter("xt", [2, s_len, B], F32, isOutput=False)
    w_firI_d = nc.declare_dram_parameter("lhsT_fir_I", [128, 24], F32, isOutput=False)
    w_firQ_d = nc.declare_dram_parameter("lhsT_fir_Q", [128, 24], F32, isOutput=False)
    w_rec_d = nc.declare_dram_parameter("lhsT_rec", [25, 44], F32, isOutput=False)
    w_out_d = nc.declare_dram_parameter("lhsT_out", [113, 8], F32, isOutput=False)
    ones_d = nc.declare_dram_parameter("ones_row", [1, (T + 1) * B], F32, isOutput=False)
    zeros_d = nc.declare_dram_parameter("zeros12", [12, B], F32, isOutput=False)
    # yt: transposed output (oc, t, b); host transposes back
    y_d = nc.declare_dram_parameter("yt", [2, s_len, B], F32, isOutput=True)

    xt_p = s_len * B              # xt/yt strides: [c, t, b] -> c*xt_p + t*B + b
    RW = (T + 1) * B              # R tile free width (slots 0..T)

    with tile.TileContext(nc) as tc:
        with (
            tc.tile_pool(name="consts", bufs=1) as cpool,
            tc.tile_pool(name="rpool", bufs=3) as rpool,
            tc.tile_pool(name="im2col", bufs=3) as impool,
            tc.tile_pool(name="psf", bufs=2, space="PSUM") as psfpool,
            tc.tile_pool(name="psr", bufs=3, space="PSUM") as psrpool,
            tc.tile_pool(name="pso", bufs=1, space="PSUM") as psopool,
            tc.tile_pool(name="demod", bufs=2) as dpool,
            tc.tile_pool(name="rec", bufs=4) as recpool,
            tc.tile_pool(name="post", bufs=2) as ppool,
        ):
            # ---- constants
            w_firI = cpool.tile([128, 24], F32)
            nc.sync.dma_start(w_firI[:], w_firI_d[:])
            w_firQ = cpool.tile([128, 24], F32)
            nc.sync.dma_start(w_firQ[:], w_firQ_d[:])
            w_rec = cpool.tile([25, 44], F32)
            nc.sync.dma_start(w_rec[:], w_rec_d[:])
            w_rec_h = cpool.tile([12, 44], F32)
            nc.sync.dma_start(w_rec_h[:], w_rec_d[0:12, :])
            w_rec_L = cpool.tile([13, 44], F32)
            nc.sync.dma_start(w_rec_L[:], w_rec_d[12:25, :])
            w_out = cpool.tile([113, 8], F32)
            nc.sync.dma_start(w_out[:], w_out_d[:])
            h0 = cpool.tile([12, B], F32)
            nc.sync.dma_start(h0[:], zeros_d[:])

            R_prev = None
            post_todo = None

            def do_pre(c, R_c):
                """im2col + FIR + demod + L rows for chunk c. Returns demod tiles."""
                t0 = c * T
                nc.gpsimd.dma_start(R_c[24:25, :], ones_d[:, 0:RW])
                s_ch = dpool.tile([24, CW], F32, name=f"s_ch{c % 4}", tag="s_ch")
                mag_ch = dpool.tile([24, CW], F32, name=f"mag{c % 4}", tag="mag")
                cos_ch = dpool.tile([24, CW], F32, name=f"cos{c % 4}", tag="cos")
                sin_ch = dpool.tile([24, CW], F32, name=f"sin{c % 4}", tag="sin")
                for sidx in range(NSUB):
                    im = impool.tile([128, NSUBF], F32, name=f"im{c % 4}_{sidx}", tag="im")
                    pitch_im = _pitch(im)
                    tsub = t0 + sidx * SUBT
                    # --- im2col DMA: rows pair*64 + j, j = blk*TB + k
                    # (t = tsub + j + tau'' - 15; one DMA per pair)
                    tw = tsub - (WIN - 1)
                    jmin = max(0, -tw)
                    for pair in range(2):
                        if jmin > 0:
                            nc.vector.memset(
                                _ap(im[:], pair * 64 * pitch_im,
                                    [(pitch_im, 32), (1, NSUBF)]), 0.0)
                            for j in range(jmin):
                                tau_min = -(tw + j)
                                if tau_min >= SUBT:
                                    continue
                                cnt = SUBT - tau_min
                                d2 = _ap(im[:],
                                         (pair * 64 + j) * pitch_im + tau_min * B,
                                         [(pitch_im, 1), (B, cnt), (1, B)])
                                s2 = _ap(x_d[0],
                                         pair * xt_p + (tw + j + tau_min) * B,
                                         [(B * cnt, 1), (B, cnt), (1, B)])
                                nc.gpsimd.dma_start(d2, s2)
                        dst = _ap(im[:], (pair * 64 + jmin) * pitch_im,
                                  [(pitch_im, 64 - jmin), (B, SUBT), (1, B)])
                        srca = _ap(x_d[0], pair * xt_p + (tw + jmin) * B,
                                   [(B, 64 - jmin), (B, SUBT), (1, B)])
                        nc.gpsimd.dma_start(dst, srca)
                    # --- FIR matmuls -> psFI/psFQ (24, 512) each
                    psFI = psfpool.tile([24, NSUBF], F32, name=f"psFI{c % 4}_{sidx}", tag="psFI")
                    nc.tensor.matmul(psFI[:], w_firI[:], im[:], start=True, stop=True)
                    psFQ = psfpool.tile([24, NSUBF], F32, name=f"psFQ{c % 4}_{sidx}", tag="psFQ")
                    nc.tensor.matmul(psFQ[:], w_firQ[:], im[:], start=True, stop=True)
                    # --- demod
                    sqI = recpool.tile([24, NSUBF], F32, name=f"sqI{c % 4}_{sidx}", tag="sqI")
                    nc.scalar.activation(sqI[:], psFI[:], AF.Square)
                    sqQ = recpool.tile([24, NSUBF], F32, name=f"sqQ{c % 4}_{sidx}", tag="sqQ")
                    nc.scalar.activation(sqQ[:], psFQ[:], AF.Square)
                    cw0 = sidx * NSUBF
                    s_sl = s_ch[:, cw0:cw0 + NSUBF]
                    nc.vector.tensor_tensor(s_sl, sqI[:], sqQ[:], OP.add)
                    lns = recpool.tile([24, NSUBF], F32, name=f"lns{c % 4}_{sidx}", tag="lns")
                    nc.scalar.activation(lns[:], s_sl, AF.Ln)
                    inv = recpool.tile([24, NSUBF], F32, name=f"inv{c % 4}_{sidx}", tag="inv")
                    nc.scalar.activation(inv[:], lns[:], AF.Exp, scale=-0.5)
                    nc.vector.tensor_tensor(mag_ch[:, cw0:cw0 + NSUBF], s_sl, inv[:], OP.mult)
                    nc.vector.tensor_tensor(cos_ch[:, cw0:cw0 + NSUBF], psFI[:], inv[:], OP.mult)
                    nc.vector.tensor_tensor(sin_ch[:, cw0:cw0 + NSUBF], psFQ[:], inv[:], OP.mult)
                # --- L rows into R_c: rows 12-17 mag, 18-23 mag2(=s)
                pr = _pitch(R_c)
                pm = _pitch(mag_ch)
                for rows0, srct in ((12, mag_ch), (18, s_ch)):
                    for blk in range(NB):
                        dst = _ap(R_c[:], rows0 * pr + blk * TB * B,
                                  [(pr, 6), (1, CW)])
                        srcb = _ap(srct[:], blk * 6 * pm, [(pm, 6), (1, CW)])
                        nc.gpsimd.dma_start(dst, srcb)
                # --- slot-0 L at base partition 0 (matmul base-partition rule)
                L0 = dpool.tile([13, B], F32, name=f"L0_{c % 4}", tag="L0")
                nc.gpsimd.dma_start(L0[12:13, :], ones_d[:, 0:B])
                nc.gpsimd.dma_start(L0[0:6, :], mag_ch[0:6, 0:B])
                nc.gpsimd.dma_start(L0[6:12, :], s_ch[0:6, 0:B])
                return cos_ch, sin_ch, L0

            def do_rec(c, R_c, R_pm1, L0):
                """recurrence steps for chunk c."""
                for sl in range(T):
                    psR = psrpool.tile([44, B], F32, name=f"psR{sl % 8}", tag="psR")
                    if sl == 0:
                        h_prev = h0[:] if c == 0 else R_pm1[0:12, T * B:(T + 1) * B]
                        nc.tensor.matmul(psR[:], w_rec_h[:], h_prev,
                                         start=True, stop=False)
                        nc.tensor.matmul(psR[:], w_rec_L[:],
                                         L0[:], start=False, stop=True)
                    else:
                        h_prev = R_c[0:12, sl * B:(sl + 1) * B]
                        nc.tensor.matmul(psR[:], w_rec[:],
                                         R_c[0:25, sl * B:(sl + 1) * B],
                                         start=True, stop=True)
                    Y = recpool.tile([44, B], F32, name=f"Y{sl % 8}", tag="Y")
                    nc.scalar.activation(Y[:], psR[:], AF.Sigmoid)
                    G = recpool.tile([12, B], F32, name=f"G{sl % 8}", tag="G")
                    nc.vector.tensor_scalar(G[:], Y[32:44, :], 2.0, -1.0, OP.mult, OP.add)
                    D = recpool.tile([12, B], F32, name=f"D{sl % 8}", tag="D")
                    nc.vector.tensor_tensor(D[:], h_prev, G[:], OP.subtract)
                    M = recpool.tile([12, B], F32, name=f"M{sl % 8}", tag="M")
                    nc.vector.tensor_tensor(M[:], Y[0:12, :], D[:], OP.mult)
                    nc.vector.tensor_tensor(
                        R_c[0:12, (sl + 1) * B:(sl + 2) * B], G[:], M[:], OP.add)

            def do_post(c, R_c, cos_ch, sin_ch):
                """phase rotation + output projection + store for chunk c."""
                pr = _pitch(R_c)
                hB = ppool.tile([48, CW], F32, name=f"hB{c % 4}", tag="hB")
                ph = _pitch(hB)
                # h_seq: R_c rows 0-11 slots 1..T  ->  (blk*12+ch, tau*64+b)
                for blk in range(NB):
                    nc.gpsimd.dma_start(
                        _ap(hB[:], blk * 12 * ph, [(ph, 12), (1, CW)]),
                        _ap(R_c[:], (1 + blk * TB) * B, [(pr, 12), (1, CW)]))
                CC = ppool.tile([48, CW], F32, name=f"CC{c % 4}", tag="CC")
                SS = ppool.tile([48, CW], F32, name=f"SS{c % 4}", tag="SS")
                pc = _pitch(CC)
                pcs = _pitch(cos_ch)
                for blk in range(NB):
                    for half in range(2):
                        nc.gpsimd.dma_start(
                            _ap(CC[:], (blk * 12 + half * 6) * pc,
                                [(pc, 6), (1, CW)]),
                            _ap(cos_ch[:], blk * 6 * pcs, [(pcs, 6), (1, CW)]))
                        nc.gpsimd.dma_start(
                            _ap(SS[:], (blk * 12 + half * 6) * pc,
                                [(pc, 6), (1, CW)]),
                            _ap(sin_ch[:], blk * 6 * pcs, [(pcs, 6), (1, CW)]))
                O = ppool.tile([113, CW], F32, name=f"O{c % 4}", tag="O")
                nc.gpsimd.dma_start(O[112:113, :], ones_d[:, 0:CW])
                nc.vector.tensor_tensor(O[0:48, :], hB[:], CC[:], OP.mult)
                nc.vector.tensor_tensor(O[64:112, :], hB[:], SS[:], OP.mult)
                outsb = ppool.tile([8, CW], F32, name=f"outsb{c % 4}", tag="outsb")
                for n in range(CW // NSUBF):
                    psO = psopool.tile([8, NSUBF], F32, name=f"psO{n % 2}", tag="psO")
                    nc.tensor.matmul(psO[:], w_out[:],
                                     O[:, n * NSUBF:(n + 1) * NSUBF],
                                     start=True, stop=True)
                    nc.scalar.copy(outsb[:, n * NSUBF:(n + 1) * NSUBF], psO[:])
                po = _pitch(outsb)
                for oc in range(2):
                    nc.gpsimd.dma_start(
                        _ap(y_d[0], oc * xt_p + c * T * B, [(NB * CW, 1), (1, NB * CW)]),
                        _ap(outsb[:], oc * po, [(2 * po, NB), (1, CW)]))

            for c in range(nch):
                R_c = rpool.tile([25, RW], F32, name=f"R{c % 3}", tag="R")
                cos_ch, sin_ch, L0 = do_pre(c, R_c)
                do_rec(c, R_c, R_prev, L0)
                if post_todo is not None:
                    do_post(*post_todo)
                post_todo = (c, R_c, cos_ch, sin_ch)
                R_prev = R_c
            do_post(*post_todo)

    split_waits(nc)
    return nc


# ---------------- host-side execution ----------------

_CACHE = {}


def _get_exec(s_len):
    """Build + jit once; returns runner(in_maps) -> list[dict] per core."""
    if s_len in _CACHE:
        return _CACHE[s_len]
    import jax
    import jax.numpy  # noqa: F401
    from jax.sharding import Mesh, PartitionSpec
    from jax.experimental.shard_map import shard_map
    from concourse import bass2jax
    from concourse import mybir as _mb

    nc = build_nc(s_len)
    bass2jax.install_neuronx_cc_hook()

    in_names, out_names, out_avals, zero_shapes = [], [], [], []
    partition_name = (nc.partition_id_tensor.name
                      if nc.partition_id_tensor else None)
    for alloc in nc.m.functions[0].allocations:
        if not isinstance(alloc, _mb.MemoryLocationSet):
            continue
        name = alloc.memorylocations[0].name
        if alloc.kind == "ExternalInput":
            if name != partition_name:
                in_names.append(name)
        elif alloc.kind == "ExternalOutput":
            shape = tuple(alloc.tensor_shape)
            dtype = _mb.dt.np(alloc.dtype)
            out_names.append(name)
            out_avals.append(jax.core.ShapedArray(shape, dtype))
            zero_shapes.append((shape, dtype))
    n_params = len(in_names)
    n_outs = len(out_names)
    all_names = list(in_names) + list(out_names)
    if partition_name is not None:
        all_names.append(partition_name)
    donate = tuple(range(n_params, n_params + n_outs))

    def _body(*args):
        operands = list(args)
        if partition_name is not None:
            operands.append(bass2jax.partition_id_tensor())
        outs = bass2jax._bass_exec_p.bind(
            *operands,
            out_avals=tuple(out_avals),
            in_names=tuple(all_names),
            out_names=tuple(out_names),
            lowering_input_output_aliases=(),
            sim_require_finite=True,
            sim_require_nnan=True,
            nc=nc,
        )
        return tuple(outs)

    devices = jax.devices()[:NCORES]
    mesh = Mesh(np.asarray(devices), ("core",))
    in_specs = (PartitionSpec("core"),) * (n_params + n_outs)
    out_specs = (PartitionSpec("core"),) * n_outs
    sharded = jax.jit(
        shard_map(_body, mesh=mesh, in_specs=in_specs, out_specs=out_specs,
                  check_rep=False),
        donate_argnums=donate, keep_unused=True)

    def runner(in_maps):
        concat_in = [
            np.concatenate([np.asarray(in_maps[c][nm]) for c in range(NCORES)],
                           axis=0)
            for nm in in_names]
        concat_zeros = [np.zeros((NCORES * sh[0],) + sh[1:], dt)
                        for sh, dt in zero_shapes]
        out_arrs = sharded(*concat_in, *concat_zeros)
        return [
            {nm: np.asarray(out_arrs[i]).reshape((NCORES,) + zero_shapes[i][0])[c]
             for i, nm in enumerate(out_names)}
            for c in range(NCORES)]

    runner.sharded = sharded
    runner.in_names = in_names
    runner.out_names = out_names
    runner.zero_shapes = zero_shapes
    runner.mesh = mesh
    _CACHE[s_len] = runner
    return runner


def kernel(**inputs):
    x = np.ascontiguousarray(np.asarray(inputs["x"], np.float32))
    bt, s_len, _ = x.shape
    assert bt == B_TOT and s_len == S, (bt, s_len)
    dw = derive_weights(inputs)
    runner = _get_exec(s_len)

    in_maps = []
    for c in range(NCORES):
        xt = np.ascontiguousarray(
            x[c * B:(c + 1) * B].transpose(2, 1, 0))   # (2, S, B)
        m = {"xt": xt}
        m.update(dw)
        in_maps.append(m)
    results = runner(in_maps)
    out = np.concatenate(
        [results[c]["yt"].transpose(2, 1, 0) for c in range(NCORES)], axis=0)
    return np.ascontiguousarray(out, np.float32)

